# revision 23
# baseline (speedup 1.0000x reference)
"""MoE top-2 SwiGLU FFN kernel for 8 Trainium2 NeuronCores.

Strategy: host-side gating/dispatch + 8-way tensor-parallel experts.

  - The gate (x @ Wg, top-2, softmax) is tiny; computed on the host in
    float64 and used to dispatch tokens to experts ("all-to-all by
    top-k indices" done during input sharding).
  - Hidden-dim tensor parallelism over all experts: core c owns the
    hidden-slice [c*H/8, (c+1)*H/8) of EVERY expert.  Each core
    processes ALL routed (token, expert) rows -- perfect load balance,
    zero padding waste -- and produces a partial Y (its hidden slice's
    contribution), which the host sums across the 8 cores.
  - Per (expert e, hidden 128-chunk j) on each core:
        Ht1 = W1[e][:, slice].T-chunks @ Xt_e   (accumulated over D)
        Ht3 = W3[e][:, slice].T-chunks @ Xt_e
        Gt  = silu(Ht1) * Ht3                   kept in SBUF (bf16)
    then Yt_partial[dt] = sum_j W2-chunks.T @ Gt  per 128-wide d-tile.
    Tokens always live in the matmul moving/free dimension, so per-expert
    token counts need no 128-alignment and no on-chip transposes exist.
  - Host applies the softmax combine weights and scatter-adds the two
    expert contributions per token.

  Matmuls run in bf16 (fp32 PSUM accumulation). Weights are pre-packed
  on the host so every DMA has >=1KB contiguous per-partition rows.
  SBUF tiles use a 64-token-aligned pitch (128B rows) for full-rate PE
  streaming; only the real token count is computed.
"""

import numpy as np
import ml_dtypes

E = 8          # experts
D = 2048       # model dim
H = 7168       # hidden dim
P = 128
DK = D // P    # 16 contraction chunks for stage 1
HJ = (H // E) // P   # 7 hidden 128-chunks per expert per core
HK = E * HJ    # 56 hk-tiles per core (e, j)
DT = D // P    # 16 output d-tiles for stage 2
TOP_K = 2

BF16 = ml_dtypes.bfloat16

_built_cache = {}


def _chunks(c):
    """Split token count c into matmul moving-dim chunks (<=512 each)."""
    out = []
    c0 = 0
    while c0 < c:
        cn = min(512, c - c0)
        out.append((c0, cn))
        c0 += cn
    return out


def _pad64(c):
    return -(-c // 64) * 64


def _build(counts):
    """Build + compile the SPMD Bass program for per-expert token counts.

    Tries buffer configs from fastest to most frugal so unusual token
    distributions (bigger tiles) still compile, just with less
    double-buffering."""
    key = tuple(counts)
    if key in _built_cache:
        return _built_cache[key]
    last = None
    for bufs in ((2, 4, 3, 8), (2, 3, 2, 4), (1, 2, 2, 3), (1, 1, 1, 2)):
        try:
            nc = _build_with(counts, *bufs)
            _built_cache[key] = nc
            return nc
        except ValueError as err:
            last = err
    raise last


def _build_with(counts, x_bufs, w_bufs, w2_bufs, y_bufs):

    import concourse.tile as tile
    import concourse.mybir as mybir
    from concourse import bacc
    from contextlib import ExitStack

    bf16 = mybir.dt.bfloat16
    f32 = mybir.dt.float32
    f16 = mybir.dt.float16

    pads = [_pad64(c) for c in counts]
    offs = np.concatenate([[0], np.cumsum(pads)]).tolist()
    CE = offs[-1]          # padded total token columns
    CPS = max(576, max(pads))       # PSUM tile width (fp32)
    ps_banks = -(-CPS * 4 // 2048)  # PSUM banks per tile
    ps1_bufs = 2 if ps_banks <= 2 else 1
    ps2_bufs = 4 if ps_banks <= 2 else 1

    nc = bacc.Bacc("TRN2", target_bir_lowering=False, debug=False)

    xt = nc.dram_tensor("xt", [DK, P, CE], bf16, kind="ExternalInput").ap()
    w1 = nc.dram_tensor("w1", [HK, P, DK, P], bf16, kind="ExternalInput").ap()
    w3 = nc.dram_tensor("w3", [HK, P, DK, P], bf16, kind="ExternalInput").ap()
    w2 = nc.dram_tensor("w2", [DT, P, HK, P], bf16, kind="ExternalInput").ap()
    yt = nc.dram_tensor("yt", [DT, P, CE], f16, kind="ExternalOutput").ap()

    with tile.TileContext(nc) as tc, ExitStack() as ctx:
        xpool = ctx.enter_context(tc.tile_pool(name="xpool", bufs=x_bufs))
        gpool = ctx.enter_context(tc.tile_pool(name="gpool", bufs=1))
        wpool = ctx.enter_context(tc.tile_pool(name="wpool", bufs=w_bufs))
        spool = ctx.enter_context(tc.tile_pool(name="spool", bufs=3))

        # PE pre-warm: dummy matmuls on a memset tile keep the PE busy
        # (and get the HAM clock-gate to 8/8) while the first real
        # weight/activation DMAs are still in flight.
        warm = ctx.enter_context(tc.tile_pool(name="warm", bufs=1))
        wsrc = warm.tile([P, 512], bf16, name="wsrc", tag="wsrc")
        with tc.tile_pool(name="pwarm", bufs=1, space="PSUM") as pwarm:
            wdst = pwarm.tile([P, 512], f32, name="wdst", tag="wdst")
            nc.gpsimd.memset(wsrc[:], 0.0)
            for _ in range(24):
                nc.tensor.matmul(wdst[:], wsrc[:, :P], wsrc[:], start=True,
                                 stop=True)

        def load_w13(t):
            w1t = wpool.tile([P, DK, P], bf16, name=f"w1t{t}", tag="w1t")
            w3t = wpool.tile([P, DK, P], bf16, name=f"w3t{t}", tag="w3t")
            nc.sync.dma_start(out=w1t[:, :DK // 2, :],
                              in_=w1[t][:, :DK // 2, :])
            nc.sync.dma_start(out=w3t[:, :DK // 2, :],
                              in_=w3[t][:, :DK // 2, :])
            nc.sync.dma_start(out=w1t[:, DK // 2:, :],
                              in_=w1[t][:, DK // 2:, :])
            nc.sync.dma_start(out=w3t[:, DK // 2:, :],
                              in_=w3[t][:, DK // 2:, :])
            return w1t, w3t

        # Stage 1: per (expert, hidden chunk): Gt = silu(Ht1) * Ht3
        gts = []
        xts = {}
        preload = {t: load_w13(t) for t in (0, 1)}
        with tc.tile_pool(name="ps1", bufs=ps1_bufs, space="PSUM") as ps1:
            for e in range(E):
                ce, pe_, off = counts[e], pads[e], offs[e]
                chunks = _chunks(ce)
                for dk in range(DK):
                    xtile = xpool.tile([P, pe_], bf16, name=f"xt{e}_{dk}",
                                       tag=f"xt{dk}")
                    nc.scalar.dma_start(out=xtile[:, :ce],
                                        in_=xt[dk][:, off:off + ce])
                    xts[(e, dk)] = xtile
                for j in range(HJ):
                    t = e * HJ + j
                    if t in preload:
                        w1t, w3t = preload[t]
                    else:
                        w1t, w3t = load_w13(t)

                    h1 = ps1.tile([P, CPS], f32, name=f"h1_{t}", tag="h1")
                    h3 = ps1.tile([P, CPS], f32, name=f"h3_{t}", tag="h3")
                    for dk in range(DK):
                        st = dk == 0
                        sp = dk == DK - 1
                        xk = xts[(e, dk)]
                        for (c0, cn) in chunks:
                            nc.tensor.matmul(
                                h1[:, c0:c0 + cn], w1t[:, dk, :],
                                xk[:, c0:c0 + cn], start=st, stop=sp,
                            )
                        for (c0, cn) in chunks:
                            nc.tensor.matmul(
                                h3[:, c0:c0 + cn], w3t[:, dk, :],
                                xk[:, c0:c0 + cn], start=st, stop=sp,
                            )
                    s1 = spool.tile([P, pe_], bf16, name=f"s1_{t}", tag="s1")
                    nc.scalar.activation(
                        s1[:, :ce], h1[:, :ce],
                        mybir.ActivationFunctionType.Silu,
                    )
                    g = gpool.tile([P, pe_], bf16, name=f"gt{t}", tag=f"gt{t}")
                    nc.vector.tensor_mul(g[:, :ce], s1[:, :ce], h3[:, :ce])
                    gts.append(g)

        # Stage 2: partial Yt[dt] = sum_j W2[e-slice].T-chunks @ Gt
        w2pool = ctx.enter_context(tc.tile_pool(name="w2pool", bufs=w2_bufs))
        ypool = ctx.enter_context(tc.tile_pool(name="ypool", bufs=y_bufs))
        with tc.tile_pool(name="ps2", bufs=ps2_bufs, space="PSUM") as ps2:
            for dt in range(DT):
                w2t = w2pool.tile([P, HK, P], bf16, name=f"w2t{dt}", tag="w2t")
                for q in range(4):
                    nc.sync.dma_start(out=w2t[:, q * 14:(q + 1) * 14, :],
                                      in_=w2[dt][:, q * 14:(q + 1) * 14, :])
                for e in range(E):
                    ce, pe_, off = counts[e], pads[e], offs[e]
                    chunks = _chunks(ce)
                    yp = ps2.tile([P, CPS], f32, name=f"yp{dt}_{e}", tag="yp")
                    for j in range(HJ):
                        t = e * HJ + j
                        st = j == 0
                        sp = j == HJ - 1
                        for (c0, cn) in chunks:
                            nc.tensor.matmul(
                                yp[:, c0:c0 + cn], w2t[:, t, :],
                                gts[t][:, c0:c0 + cn], start=st, stop=sp,
                            )
                    yo = ypool.tile([P, pe_], f16, name=f"yo{dt}_{e}",
                                    tag="yo")
                    if e % 2 == 0:
                        nc.scalar.copy(yo[:, :ce], yp[:, :ce])
                    else:
                        nc.vector.tensor_copy(yo[:, :ce], yp[:, :ce])
                    if dt == DT - 1 and e % 2 == 1:
                        nc.sync.dma_start(out=yt[dt][:, off:off + ce],
                                          in_=yo[:, :ce])
                    else:
                        nc.gpsimd.dma_start(out=yt[dt][:, off:off + ce],
                                            in_=yo[:, :ce])

    nc.compile()
    return nc


def kernel(x, Wg, w1, w3, w2):
    from concourse.bass_utils import run_bass_kernel_spmd

    Bs, Ss, Dd = x.shape
    T = Bs * Ss
    xf = np.ascontiguousarray(x.reshape(T, Dd).astype(np.float32))

    # ---- host gate: scores, top-2, softmax (float64 for stability) ----
    scores = xf.astype(np.float64) @ np.asarray(Wg, np.float64)
    ei = np.argpartition(-scores, TOP_K - 1, axis=1)[:, :TOP_K]
    row = np.arange(T)[:, None]
    sv = scores[row, ei]
    order = np.argsort(-sv, axis=1)
    ei = ei[row, order]                     # [T, 2] expert ids, desc score
    sv = sv[row, order]
    svm = sv - sv.max(axis=1, keepdims=True)
    esv = np.exp(svm)
    cw = (esv / esv.sum(axis=1, keepdims=True)).astype(np.float32)  # [T, 2]

    # ---- dispatch: token lists per expert ----
    idx = [np.where((ei == e).any(axis=1))[0] for e in range(E)]
    wts = []
    for e in range(E):
        sel = ei[idx[e]]
        k = (sel == e).argmax(axis=1)
        wts.append(cw[idx[e], k])
    counts = [len(i) for i in idx]
    pads = [_pad64(c) for c in counts]
    offs = np.concatenate([[0], np.cumsum(pads)]).astype(int)
    CE = int(offs[-1])

    nc = _build(counts)

    # ---- pack inputs ----
    # xt: [DK, P, CE] -- identical on every core (all routed rows).
    xall = np.zeros((CE, D), np.float32)
    for e in range(E):
        xall[offs[e]:offs[e] + counts[e]] = xf[idx[e]]
    xtp = np.ascontiguousarray(xall.T.astype(BF16)).reshape(DK, P, CE)

    HS = H // E  # per-core hidden slice width (896)
    w1a, w3a, w2a = np.asarray(w1), np.asarray(w3), np.asarray(w2)
    in_maps = []
    for c in range(E):
        # stage-1 weights: hk-tile t = e*HJ + j covers expert e's hidden
        # rows [c*HS + j*128, ...+128).  Packed [t, p(d), dk, h] so each
        # hk-tile DMA has 4KB contiguous rows.
        sl = slice(c * HS, (c + 1) * HS)
        w1p = np.ascontiguousarray(
            w1a[:, :, sl].astype(BF16)              # [E, D, HS]
            .reshape(E, DK, P, HJ, P)                # [e, dk, p, j, h]
            .transpose(0, 3, 2, 1, 4)                # [e, j, p, dk, h]
            .reshape(HK, P, DK, P)
        )
        w3p = np.ascontiguousarray(
            w3a[:, :, sl].astype(BF16)
            .reshape(E, DK, P, HJ, P)
            .transpose(0, 3, 2, 1, 4)
            .reshape(HK, P, DK, P)
        )
        # stage-2 weights: [dt, p(h), t, d] with 14KB contiguous rows.
        w2p = np.ascontiguousarray(
            w2a[:, sl, :].astype(BF16)               # [E, HS, D]
            .reshape(E, HJ, P, DT, P)                # [e, j, p, dt, d]
            .transpose(3, 2, 0, 1, 4)                # [dt, p, e, j, d]
            .reshape(DT, P, HK, P)
        )
        in_maps.append({"xt": xtp, "w1": w1p, "w3": w3p, "w2": w2p})

    import os
    trace = bool(os.environ.get("MOE_TRACE"))
    res = run_bass_kernel_spmd(nc, in_maps, list(range(E)), trace=trace)
    global _last_results
    _last_results = res

    # ---- reduce partial Y across cores, combine per token ----
    ysum = res.results[0]["yt"].astype(np.float32)
    for c in range(1, E):
        ysum += res.results[c]["yt"]
    ysum = ysum.reshape(D, CE)

    y = np.zeros((T, D), np.float32)
    for e in range(E):
        ye = ysum[:, offs[e]:offs[e] + counts[e]].T
        y[idx[e]] += wts[e][:, None] * ye
    return y.reshape(Bs, Ss, Dd).astype(x.dtype)
